# revision 2
# baseline (speedup 1.0000x reference)
"""MoE expert FFN (forward_all + top-2 routing combine) on 8 TRN2 NeuronCores.

Strategy: the routing tensor has exactly TOP_K=2 nonzeros per token, so only
routed (token, expert) pairs contribute. We dispatch on the host and run the
expert GEMMs expert-parallel on 8 cores. The device program is a sequence of
K "phases"; each phase is a (weights, tokens) GEMM pair of a fixed compiled
capacity, and the host assigns any expert's weights + routed tokens to any
(core, phase) slot. Because the program is compiled per observed routing
counts (cached by capacity signature), experts can be SPLIT across slots and
capacities chosen so padding is ~1-2% instead of padding every expert to a
global cap. A small solver picks piece cuts/capacities minimizing
sum-of-group-maxima.

Each phase computes y^T = gate * (w2^T @ gelu(w1^T @ x^T + b1)) with tokens
on the matmul free dim, f16 matmuls (f32 PSUM accumulate, bias+gelu in f32,
gate-mul on DVE), k-interleaved over 8 PSUM banks. Weight k-slices stream
column-half-first in consumption order; the next phase's weights dep-chain
behind the current phase's so prefetch never starves the critical path.
Input DMAs issue from the Scalar-engine DGE, weights from Sync, splitting
descriptor-issue cost across both HWDGE queues. A short warmup matmul block
runs during the initial DMA wait to ramp the PE p-state. y is written f16.
"""

import math
from contextlib import ExitStack

import numpy as np

import concourse.mybir as mybir
import concourse.tile as tile
from concourse import bacc
from concourse.bass_utils import run_bass_kernel_spmd

N, DIM, E, EXPERT_DIM = 8192, 1024, 16, 2048
N_CORES = 8
P = 128

KO1 = DIM // P  # 8 contraction tiles, stage 1
MO1 = EXPERT_DIM // P  # 16 output tiles, stage 1
KO2 = EXPERT_DIM // P  # 16 contraction tiles, stage 2
MO2 = DIM // P  # 8 output tiles, stage 2

GRP = 8  # stage-1 psum group = all 8 PSUM banks
MIN_CHUNK = 240  # below this, ldweights (~97ns) outruns the matmul free dim

TRACE = False  # set by test.py to capture an NTFF profile
LAST_EXEC_NS = None
LAST_TRACE_PATH = None
ACT_FUNC = None  # default Gelu; sim_check overrides (CoreSim lacks Gelu)
WARMUP_MM = 12  # matmuls ramping the PE p-state during the startup DMA wait

_NC_CACHE = {}


def _chunks_for(cap, last_phase):
    """Split a phase capacity into matmul chunks <=512 (one PSUM bank of f32),
    each >=MIN_CHUNK when possible. The very last chunk of the last phase is
    kept small-ish so the post-matmul tail (gate-mul + y DMA) is short."""
    if cap <= 512:
        return [cap]
    n = -(-cap // 512)  # ceil
    while True:
        base = cap // n
        if base >= MIN_CHUNK or n == 2:
            break
        n -= 1
    ch = [base + (1 if i < cap % n else 0) for i in range(n)]
    ch.sort(reverse=True)
    if last_phase and len(ch) > 1:
        # move a ~256 tail to the end
        tail = min(ch[-1], 256)
        rest = cap - tail
        m = len(ch) - 1
        ch = [rest // m + (1 if i < rest % m else 0) for i in range(m)]
        ch.sort(reverse=True)
        ch.append(tail)
    return ch


def _build_nc(caps):
    """caps: tuple of per-phase token capacities (descending-ish)."""
    f32 = mybir.dt.float32
    f16 = mybir.dt.float16
    K = len(caps)

    nc = bacc.Bacc("TRN2", target_bir_lowering=False, debug=False, num_devices=N_CORES)
    xts = [
        nc.dram_tensor(f"xt{k}", [DIM, caps[k]], f16, kind="ExternalInput").ap()
        for k in range(K)
    ]
    w1 = nc.dram_tensor("w1", [K, DIM, EXPERT_DIM], f16, kind="ExternalInput").ap()
    b1 = nc.dram_tensor("b1", [K, P, MO1], f32, kind="ExternalInput").ap()
    w2 = nc.dram_tensor("w2", [K, EXPERT_DIM, DIM], f16, kind="ExternalInput").ap()
    gts = [
        nc.dram_tensor(f"g{k}", [P, caps[k]], f32, kind="ExternalInput").ap()
        for k in range(K)
    ]
    yts = [
        nc.dram_tensor(f"yt{k}", [DIM, caps[k]], f16, kind="ExternalOutput").ap()
        for k in range(K)
    ]

    gelu = ACT_FUNC or mybir.ActivationFunctionType.Gelu
    half_cols = (MO1 // 2) * P

    with tile.TileContext(nc) as tc, ExitStack() as ctx:
        w1_pool = ctx.enter_context(tc.tile_pool(name="w1", bufs=KO1 + 6))
        w2_pool = ctx.enter_context(tc.tile_pool(name="w2", bufs=KO2 + 6))
        b1_pool = ctx.enter_context(tc.tile_pool(name="b1", bufs=2))
        x_pool = ctx.enter_context(tc.tile_pool(name="x", bufs=4))
        g_pool = ctx.enter_context(tc.tile_pool(name="g", bufs=3))
        h_pool = ctx.enter_context(tc.tile_pool(name="h", bufs=2))
        y_pool = ctx.enter_context(tc.tile_pool(name="y", bufs=6))
        wu_pool = ctx.enter_context(tc.tile_pool(name="wu", bufs=1))
        ps_pool = ctx.enter_context(tc.tile_pool(name="ps", bufs=GRP, space="PSUM"))

        # PE warmup: ramp the tensor-engine p-state while the first weight/x
        # DMAs are in flight. Garbage values; the psum tile is consumed by a
        # throwaway copy so the chain has a reader.
        wu_t = wu_pool.tile([P, 256], f16)
        nc.vector.memset(wu_t[:], 0.0)
        wu_ps = ps_pool.tile([P, 256], f32, tag="ps", name="wu_ps")
        for i in range(WARMUP_MM):
            nc.tensor.matmul(
                wu_ps[:],
                wu_t[:, :P],
                wu_t[:],
                start=(i == 0),
                stop=(i == WARMUP_MM - 1),
            )
        nc.vector.tensor_copy(wu_t[:], wu_ps[:])

        w_phase_gate = None  # last weight DMA of the previous phase
        for k in range(K):
            cap = caps[k]
            chunks = _chunks_for(cap, k == K - 1)
            n_ch = len(chunks)
            offs = [sum(chunks[:i]) for i in range(n_ch)]

            x_ts = []
            g_ts = []
            # --- weight + input DMA issue for this phase ---
            # stage-1 consumption order: half A (cols 0..half) of w1 slices
            # ko=0..7 with x k-slices, then half B. w2 streams after w1.
            xt_r = xts[k].rearrange("(ko p) n -> p ko n", p=P)
            w1_sl = []
            wA = []
            for ko in range(KO1):
                w = w1_pool.tile([P, EXPERT_DIM], f16, tag="w1")
                da = nc.sync.dma_start(
                    w[:, :half_cols], w1[k, ko * P : (ko + 1) * P, :half_cols]
                )
                if w_phase_gate is not None:
                    tile.add_dep_helper(da.ins, w_phase_gate, reason="phase order")
                wA.append(da)
                w1_sl.append(w)
            # first chunk's x: ko0 alone (critical), then the rest in one DMA
            x0 = x_pool.tile([P, KO1, chunks[0]], f16, tag="x", name=f"x_{k}_0")
            dx0 = nc.scalar.dma_start(x0[:, 0], xt_r[:, 0, : chunks[0]])
            if w_phase_gate is not None:
                tile.add_dep_helper(dx0.ins, w_phase_gate, reason="phase order")
            dxr = nc.scalar.dma_start(x0[:, 1:], xt_r[:, 1:, : chunks[0]])
            tile.add_dep_helper(dxr.ins, wA[0].ins, reason="x rest after first w")
            x_ts.append(x0)
            last_w1 = None
            for ko in range(KO1):
                w = w1_sl[ko]
                d = nc.sync.dma_start(
                    w[:, half_cols:], w1[k, ko * P : (ko + 1) * P, half_cols:]
                )
                if w_phase_gate is not None:
                    tile.add_dep_helper(d.ins, w_phase_gate, reason="phase order")
                last_w1 = d
            b1_t = b1_pool.tile([P, MO1], f32)
            nc.scalar.dma_start(b1_t[:], b1[k])
            # later chunks' x + gates chain behind this phase's w1 stream
            for t in range(1, n_ch):
                x_t = x_pool.tile([P, KO1, chunks[t]], f16, tag="x", name=f"x_{k}_{t}")
                d = nc.scalar.dma_start(
                    x_t[:], xt_r[:, :, offs[t] : offs[t] + chunks[t]]
                )
                tile.add_dep_helper(d.ins, last_w1.ins, reason="x after w1")
                x_ts.append(x_t)
            for t in range(n_ch):
                g_t = g_pool.tile([P, chunks[t]], f32, tag="g", name=f"g_{k}_{t}")
                dg = nc.scalar.dma_start(g_t[:], gts[k][:, offs[t] : offs[t] + chunks[t]])
                tile.add_dep_helper(dg.ins, last_w1.ins, reason="g after w1")
                g_ts.append(g_t)
            w2_sl = []
            for ko in range(KO2):
                w = w2_pool.tile([P, DIM], f16, tag="w2")
                d = nc.sync.dma_start(w[:], w2[k, ko * P : (ko + 1) * P, :])
                tile.add_dep_helper(d.ins, last_w1.ins, reason="w2 behind w1")
                w2_sl.append(w)
            w_phase_gate = d.ins

            # --- compute ---
            for t in range(n_ch):
                tok = chunks[t]
                tsl = slice(offs[t], offs[t] + tok)
                x_t = x_ts[t]
                g_t = g_ts[t]

                # stage 1: h^T = gelu(w1^T @ x^T + b1), k-interleaved
                h_t = h_pool.tile([P, MO1, tok], f16, tag="h", name=f"h_{k}_{t}")
                for half in range(MO1 // GRP):
                    pss = [
                        ps_pool.tile(
                            [P, tok], f32, tag="ps", name=f"ps_{k}_{t}_{half}_{i}"
                        )
                        for i in range(GRP)
                    ]
                    for ko in range(KO1):
                        for i in range(GRP):
                            mo = half * GRP + i
                            nc.tensor.matmul(
                                pss[i][:],
                                w1_sl[ko][:, mo * P : (mo + 1) * P],
                                x_t[:, ko],
                                start=(ko == 0),
                                stop=(ko == KO1 - 1),
                            )
                    for i in range(GRP):
                        mo = half * GRP + i
                        nc.scalar.activation(
                            h_t[:, mo], pss[i][:], gelu, bias=b1_t[:, mo : mo + 1]
                        )

                # stage 2: y^T = gate * (w2^T @ h^T)
                last = k == K - 1 and t == n_ch - 1
                if last:
                    # m-outer so early m tiles' gate-mul + y DMA overlap the
                    # remaining matmuls (short tail)
                    for mo in range(MO2):
                        ps2 = ps_pool.tile(
                            [P, tok], f32, tag="ps", name=f"ps2_{k}_{t}_{mo}"
                        )
                        for ko in range(KO2):
                            nc.tensor.matmul(
                                ps2[:],
                                w2_sl[ko][:, mo * P : (mo + 1) * P],
                                h_t[:, ko],
                                start=(ko == 0),
                                stop=(ko == KO2 - 1),
                            )
                        y_t = y_pool.tile([P, tok], f16, tag="y", name=f"y_{k}_{t}_{mo}")
                        nc.vector.tensor_mul(y_t[:], ps2[:], g_t[:])
                        nc.sync.dma_start(yts[k][mo * P : (mo + 1) * P, tsl], y_t[:])
                else:
                    G2 = MO2 // 2
                    for half2 in range(2):
                        pss2 = [
                            ps_pool.tile(
                                [P, tok], f32, tag="ps", name=f"ps2_{k}_{t}_{half2}_{i}"
                            )
                            for i in range(G2)
                        ]
                        for ko in range(KO2):
                            for i in range(G2):
                                mo = half2 * G2 + i
                                nc.tensor.matmul(
                                    pss2[i][:],
                                    w2_sl[ko][:, mo * P : (mo + 1) * P],
                                    h_t[:, ko],
                                    start=(ko == 0),
                                    stop=(ko == KO2 - 1),
                                )
                        for i in range(G2):
                            mo = half2 * G2 + i
                            y_t = y_pool.tile(
                                [P, tok], f16, tag="y", name=f"y_{k}_{t}_{mo}"
                            )
                            nc.vector.tensor_mul(y_t[:], pss2[i][:], g_t[:])
                            nc.sync.dma_start(
                                yts[k][mo * P : (mo + 1) * P, tsl], y_t[:]
                            )

    nc.compile()
    return nc


def _get_nc(caps):
    key = tuple(caps)
    if key not in _NC_CACHE:
        _NC_CACHE[key] = _build_nc(key)
    return _NC_CACHE[key]


def _solve_slots(counts):
    """Cut experts into pieces and group the 8*K pieces into K phase groups
    (8 slots each) minimizing sum of group maxima (= per-core compiled work).

    Every expert is cut into 2 pieces (K=4 groups). Cuts are chosen by a
    small local search flattening the sorted-piece group boundaries.
    Returns (caps, assignment) where assignment[core] = list over phases of
    (expert, lo, hi) token-range claims (hi-lo may be < cap -> zero-padded).
    """
    E_ = len(counts)
    K = (2 * E_) // 8  # 4 for E=16

    # initial cut: halves
    pieces = []  # (expert, size)
    for e, c in enumerate(counts):
        a = c // 2
        pieces.append([e, c - a])
        pieces.append([e, a])

    def group_cost(pl):
        s = sorted((sz for _, sz in pl), reverse=True)
        return sum(s[8 * g] for g in range(K))

    # local search: move mass between an expert's two pieces to reduce the
    # sum of group boundary maxima
    for _ in range(200):
        base = group_cost(pieces)
        improved = False
        for ei in range(E_):
            i1, i2 = 2 * ei, 2 * ei + 1
            tot = pieces[i1][1] + pieces[i2][1]
            lo = max(MIN_CHUNK, tot - 8 * 512)
            for a in range(max(lo, tot // 2 - 160), min(tot - lo, tot // 2 + 161), 8):
                old = (pieces[i1][1], pieces[i2][1])
                pieces[i1][1], pieces[i2][1] = tot - a, a
                c = group_cost(pieces)
                if c < base:
                    base = c
                    improved = True
                else:
                    pieces[i1][1], pieces[i2][1] = old
        if not improved:
            break

    # group by size rank
    order = sorted(range(len(pieces)), key=lambda i: -pieces[i][1])
    caps = []
    assign = [[None] * K for _ in range(N_CORES)]
    offsets = [0] * E_  # running token offset per expert
    # stable piece->(core,phase): rank r -> phase r//8, core r%8
    # fill per expert in order so (lo,hi) ranges are consistent
    slot_of = {}
    for r, pi in enumerate(order):
        slot_of[pi] = (r % 8, r // 8)
        caps.append(pieces[pi][1])
    caps = [max(pieces[order[8 * g]][1], MIN_CHUNK) for g in range(K)]
    for pi, (e, sz) in enumerate(pieces):
        core, ph = slot_of[pi]
        lo = offsets[e]
        offsets[e] = lo + sz
        assign[core][ph] = (e, lo, lo + sz)
    return caps, assign


def _install_ntff_hook():
    """Register the axon NTFF profile hook if the image's antenv lacks it."""
    import sys
    import types

    try:
        from antenv.axon_hooks import get_axon_ntff_profile_hook  # noqa: F401

        return True
    except ImportError:
        pass
    try:
        from trn_agent_boot.trn_boot import _ntff_profile_via_ctypes

        hook = _ntff_profile_via_ctypes("/opt/axon/libaxon_pjrt.so")
        if hook is None:
            return False
        mod = types.ModuleType("antenv.axon_hooks")
        state = {"hook": hook}
        mod.set_axon_ntff_profile_hook = lambda h: state.__setitem__("hook", h)
        mod.get_axon_ntff_profile_hook = lambda: state["hook"]
        sys.modules["antenv.axon_hooks"] = mod
        return True
    except Exception:
        return False


def kernel(x, routing_tensor, w1, b1, w2):
    global LAST_EXEC_NS, LAST_TRACE_PATH
    x = np.ascontiguousarray(np.asarray(x, np.float32))
    routing_tensor = np.asarray(routing_tensor, np.float32)
    w1 = np.asarray(w1, np.float32)
    b1 = np.asarray(b1, np.float32)
    w2 = np.asarray(w2, np.float32)

    idx_list = [np.nonzero(routing_tensor[:, e])[0] for e in range(E)]
    counts = [len(i) for i in idx_list]
    caps, assign = _solve_slots(counts)
    K = len(caps)

    x16 = x.astype(np.float16)
    w1_16 = w1.astype(np.float16)
    w2_16 = w2.astype(np.float16)

    in_maps = []
    for c in range(N_CORES):
        m = {
            "w1": np.zeros((K, DIM, EXPERT_DIM), np.float16),
            "b1": np.zeros((K, P, MO1), np.float32),
            "w2": np.zeros((K, EXPERT_DIM, DIM), np.float16),
        }
        for k in range(K):
            cap = caps[k]
            xt = np.zeros((DIM, cap), np.float16)
            g = np.zeros((P, cap), np.float32)
            slot = assign[c][k]
            if slot is not None:
                e, lo, hi = slot
                idx = idx_list[e][lo:hi]
                xt[:, : hi - lo] = x16[idx].T
                g[:, : hi - lo] = routing_tensor[idx, e][None, :]
                m["w1"][k] = w1_16[e]
                m["w2"][k] = w2_16[e]
                m["b1"][k] = b1[e].reshape(MO1, P).T
            m[f"xt{k}"] = xt
            m[f"g{k}"] = g
        in_maps.append(m)

    nc = _get_nc(caps)
    core_ids = list(range(N_CORES))
    if TRACE and _install_ntff_hook():
        import concourse.bass_utils as _bu

        _bu.upload_artifacts = lambda tmpdir: tmpdir  # zero-egress container
        try:
            res = run_bass_kernel_spmd(nc, in_maps, core_ids, trace=True)
            LAST_EXEC_NS = res.exec_time_ns
            LAST_TRACE_PATH = (
                res.instructions_and_trace[1] if res.instructions_and_trace else None
            )
        except Exception:
            res = run_bass_kernel_spmd(nc, in_maps, core_ids)
    else:
        res = run_bass_kernel_spmd(nc, in_maps, core_ids)

    out = np.zeros((N, DIM), np.float32)
    for c in range(N_CORES):
        for k in range(K):
            slot = assign[c][k]
            if slot is None:
                continue
            e, lo, hi = slot
            idx = idx_list[e][lo:hi]
            yt = res.results[c][f"yt{k}"]  # [DIM, cap] f16
            out[idx] += yt[:, : hi - lo].T.astype(np.float32)

    return out
